# revision 1
# baseline (speedup 1.0000x reference)
"""Multi-head cross-attention kernel for Trainium2, 8 NeuronCores.

Problem: nn_MultiHeadAttention (H=32 heads, B=8, Lq=Lk=1024, E=128, D=512).

    keys   = einsum('bkd,hde->hbke', states, Wk) + bk
    values = einsum('bkd,hde->hbke', states, Wv) + bv
    attn   = softmax(einsum('bqe,hbke->hbqk', query, keys) / sqrt(E))
    ctx    = einsum('hbqk,hbke->hbqe', attn, values)  -> concat heads
    out    = ctx @ Wo + bo

Sharding: data parallel over batch B=8 -> one batch element per core; no
collectives needed.  Per-core dataflow (all matmuls fp32r, full PE rate):

  K^T[h] = Wk[h]-chunks @ states^T            [E=128p, Lk] psum -> SBUF (ACT),
                                              pipelined ONE HEAD AHEAD
  V[4h]  = states^T-blocks @ Wv-packed        [Lk-chunk, 4*E]  (4 heads at once)
  S^T    = K^T-block @ query^T                [Lk-chunk p, Lq] (chunked over Lk)
  P      = exp(S^T * 1/sqrt(E))               (ACT, no max-subtraction: scores
                                               are O(4) so exp is safe in fp32)
  rowsum = ones[128,128] @ P-chunks           [128, Lq] (psum accum over chunks)
  ctx^T  = V-chunk @ P-chunks                 [E, Lq]   (psum accum over chunks)
  ctxn   = ctx^T * approx_reciprocal(rowsum)  (DVE)
  out^T += Wo[h] @ ctxn                       [E, Lq]  (SBUF accumulation, DVE)

All matmuls run as float32r (fp32 rounded to 11-bit mantissa; full PE rate at
N=512, ~2.4e-4 output rel err). Two bias simplifications, both exact algebra:
 - bk is dropped entirely: its score contribution q.bk is constant over keys,
   and softmax(S + const-per-row) == softmax(S);
 - bv is folded into the output bias on the host (softmax rows sum to 1):
   bo' = bo + sum_h bv[h] @ Wo[h].
Emission order software-pipelines the PE: each head's chunk loop also carries
the NEXT head's K^T projection and the PREVIOUS head's output projection, so
the PE never waits on the DVE normalization or the ACT K-copies.
"""

import numpy as np

import concourse.bass as bass
import concourse.mybir as mybir
import concourse.tile as tile
from concourse import bacc
from concourse.bass_utils import run_bass_kernel_spmd

H, E, D = 32, 128, 512
B, LQ, LK = 8, 1024, 1024
NDC = D // 128    # 4 contraction chunks for the projections
NLK = LK // 128   # 8 key chunks
HPG = 4           # heads per group for the packed V computation
NG = H // HPG
SCALE = 1.0 / float(np.sqrt(E))

F32 = mybir.dt.float32
F32R = mybir.dt.float32r
EXP = mybir.ActivationFunctionType.Exp
COPY = mybir.ActivationFunctionType.Copy

N_CORES = 8


def _build_kernel(tc, qT, sT, wk, wv, wo, bo2, ones, outT):
    nc = tc.nc
    with (
        tc.tile_pool(name="const", bufs=1) as cpool,
        tc.tile_pool(name="wkp", bufs=2) as wkp,
        tc.tile_pool(name="wvp", bufs=2) as wvp,
        tc.tile_pool(name="wop", bufs=2) as wop,
        tc.tile_pool(name="ktp", bufs=2) as ktp,
        tc.tile_pool(name="vp", bufs=2) as vpool,
        tc.tile_pool(name="pp", bufs=4) as ppool,
        tc.tile_pool(name="normp", bufs=2) as npool,
        tc.tile_pool(name="ps_sh", bufs=2, space="PSUM") as ps_sh,
        tc.tile_pool(name="ps_acc", bufs=1, space="PSUM") as ps_acc,
    ):
        # ---- resident inputs ----
        # st is on the critical path to the first K/V matmuls; q/ones/bo2
        # are not needed until the first S chunk / rowsum / epilogue, so
        # they queue behind st
        st_sb = cpool.tile([128, NDC, LK], F32R)
        for c in range(NDC):
            nc.sync.dma_start(st_sb[:, c, :], sT[c * 128:(c + 1) * 128, :])
        q_sb = cpool.tile([E, LQ], F32R)
        ones_sb = cpool.tile([128, 128], F32R)
        bo2_sb = cpool.tile([E, 1], F32)
        out_acc = cpool.tile([E, LQ], F32)

        def emit_late_input_dmas():
            nc.sync.dma_start(q_sb[:], qT[:])
            nc.sync.dma_start(ones_sb[:], ones[:])
            nc.sync.dma_start(bo2_sb[:], bo2[:])

        # proj state carried across heads so head h+1's K/S matmuls can be
        # emitted before head h's projection (keeps PE busy during the DVE
        # normalization of head h).
        pending = {}  # h -> ctxn tile

        proj_state = {}

        def emit_proj_half(half):
            if not pending:
                return
            (h, (ctxn_sb, wo_sb)), = pending.items()
            if half == 0:
                ps_p = ps_sh.tile([E, LQ], F32, tag="sh", name="ps_p")
                proj_state.update(ps_p=ps_p)
            ps_p = proj_state["ps_p"]
            sl = bass.ts(half, 512)
            nc.tensor.matmul(ps_p[:, sl], (wo_sb[:]), (ctxn_sb[:, sl]),
                             start=True, stop=True)
            if half == 1:
                pending.clear()
                if h == 0:
                    nc.vector.tensor_scalar_add(out_acc[:], ps_p[:],
                                                bo2_sb[:, 0:1])
                else:
                    nc.vector.tensor_add(out_acc[:], out_acc[:], ps_p[:])

        def emit_pending_proj():
            emit_proj_half(0)
            emit_proj_half(1)

        kt_by_head = {}

        def emit_k(h):
            """K^T projection for head h (pipelined one head ahead).

            Two separate psum tiles per half so the ACT copy of half 0
            never WAR-serializes against the PE writing half 1.
            bk is dropped: softmax(S + const-per-row) == softmax(S)."""
            wk_sb = wkp.tile([128, NDC, E], F32R, tag="wk", name="wk_sb")
            for c in range(NDC):
                nc.sync.dma_start(wk_sb[:, c, :], wk[h, c * 128:(c + 1) * 128, :])
            kt_sb = ktp.tile([E, LK], F32R, tag="kt", name="kt_sb")
            for half in range(2):
                sl = bass.ts(half, 512)
                ps_k = ps_sh.tile([E, 512], F32, tag="sh", name="ps_k")
                for c in range(NDC):
                    nc.tensor.matmul(ps_k[:], (wk_sb[:, c, :]),
                                     (st_sb[:, c, sl]),
                                     start=(c == 0), stop=(c == NDC - 1))
                if half == 0:
                    # ACT: early psum-slot release for the S-chunk rotation
                    nc.scalar.activation(kt_sb[:, sl], ps_k[:], COPY)
                else:
                    # DVE: keeps the ACT queue free for the exps the rowsum
                    # and AV matmuls are waiting on
                    nc.vector.tensor_copy(kt_sb[:, sl], ps_k[:])
            kt_by_head[h] = kt_sb

        emit_k(0)
        emit_late_input_dmas()
        for g in range(NG):
            # ---- packed V for the 4 heads of this group ----
            wv_sb = wvp.tile([128, NDC, HPG * E], F32R, tag="wv", name="wv_sb")
            for c in range(NDC):
                nc.sync.dma_start(
                    wv_sb[:, c, :],
                    wv[c * 128:(c + 1) * 128, g * HPG * E:(g + 1) * HPG * E])
            v_sb = vpool.tile([128, NLK, HPG * E], F32R, tag="v", name="v_sb")
            for lk in range(NLK):
                ps_v = ps_sh.tile([128, HPG * E], F32, tag="sh", name="ps_v")
                for c in range(NDC):
                    nc.tensor.matmul(
                        ps_v[:], (st_sb[:, c, lk * 128:(lk + 1) * 128]),
                        (wv_sb[:, c, :]), start=(c == 0), stop=(c == NDC - 1))
                nc.scalar.activation(v_sb[:, lk, :], ps_v[:], COPY)

            for hh in range(HPG):
                h = g * HPG + hh
                kt_sb = kt_by_head.pop(h)

                # ---- attention, software-pipelined one S-chunk ahead ----
                ps_r = ps_acc.tile([128, LQ], F32, tag="r", name="ps_r")
                ps_c = ps_acc.tile([E, LQ], F32, tag="c", name="ps_c")

                def emit_s(lk, kt_sb=kt_sb):
                    ps_s = ps_sh.tile([128, LQ], F32, tag="sh", name="ps_s")
                    for half in range(2):
                        sl = bass.ts(half, 512)
                        nc.tensor.matmul(ps_s[:, sl],
                                         (kt_sb[:, lk * 128:(lk + 1) * 128]),
                                         (q_sb[:, sl]), start=True, stop=True)
                    p_sb = ppool.tile([128, LQ], F32R, tag="p", name="p_sb")
                    nc.scalar.activation(p_sb[:], ps_s[:], EXP, scale=SCALE)
                    return p_sb

                p_next = emit_s(0)
                for lk in range(NLK):
                    p_cur = p_next
                    if lk + 1 < NLK:
                        p_next = emit_s(lk + 1)
                    if lk == 0 and h + 1 < H:
                        # next head's K^T: PE work that covers this head's
                        # normalization; kt is ready well before it is needed
                        emit_k(h + 1)
                    if lk == 2:
                        # previous head's projection: by now its ctxn (DVE)
                        # and its psum slot are both long since ready
                        emit_pending_proj()
                    for half in range(2):
                        sl = bass.ts(half, 512)
                        nc.tensor.matmul(ps_r[:, sl], (ones_sb[:]),
                                         (p_cur[:, sl]),
                                         start=(lk == 0), stop=(lk == NLK - 1))
                    for half in range(2):
                        sl = bass.ts(half, 512)
                        nc.tensor.matmul(ps_c[:, sl],
                                         (v_sb[:, lk, hh * E:(hh + 1) * E]),
                                         (p_cur[:, sl]),
                                         start=(lk == 0), stop=(lk == NLK - 1))

                # reciprocal first in one op (releases ps_r for the next
                # head's first rowsum matmul as early as possible), then the
                # two multiply halves (proj half 0 only needs the first)
                recip_sb = npool.tile([128, LQ], F32, tag="recip", name="recip_sb")
                ctxn_sb = npool.tile([E, LQ], F32R, tag="ctxn", name="ctxn_sb")
                nc.vector.reciprocal_approx_fast(recip_sb[:], ps_r[:])
                for half in range(2):
                    sl = bass.ts(half, 512)
                    nc.vector.tensor_mul(ctxn_sb[:, sl], ps_c[:, sl],
                                         recip_sb[:, sl])
                wo_sb = wop.tile([E, E], F32R, tag="wo", name="wo_sb")
                nc.sync.dma_start(wo_sb[:], wo[h * E:(h + 1) * E, :])
                pending[h] = (ctxn_sb, wo_sb)

        emit_pending_proj()
        for half in range(2):
            sl = bass.ts(half, 512)
            nc.sync.dma_start(outT[:, sl], out_acc[:, sl])


def build_program():
    nc = bacc.Bacc("TRN2", target_bir_lowering=False, debug=False,
                   num_devices=N_CORES)
    qT = nc.dram_tensor("qT", [E, LQ], F32R, kind="ExternalInput").ap()
    sT = nc.dram_tensor("sT", [D, LK], F32R, kind="ExternalInput").ap()
    wk = nc.dram_tensor("wk", [H, D, E], F32R, kind="ExternalInput").ap()
    wv = nc.dram_tensor("wv", [D, H * E], F32R, kind="ExternalInput").ap()
    wo = nc.dram_tensor("wo", [H * E, E], F32R, kind="ExternalInput").ap()
    bo2 = nc.dram_tensor("bo2", [E, 1], F32, kind="ExternalInput").ap()
    ones = nc.dram_tensor("ones", [128, 128], F32R, kind="ExternalInput").ap()
    outT = nc.dram_tensor("outT", [E, LQ], F32, kind="ExternalOutput").ap()

    with tile.TileContext(nc) as tc:
        _build_kernel(tc, qT, sT, wk, wv, wo, bo2, ones, outT)
    nc.compile()
    return nc


def _round_f32r(a):
    """Round fp32 -> fp32r (11-bit mantissa, low 12 bits zero), RN-even.

    The PE's fp32r datapath keeps sign+8exp+11mantissa; the BIR verifier
    requires fp32r matmul operands to be pre-rounded, and rounding on the
    host gives round-to-nearest instead of hardware truncation.
    """
    b = np.ascontiguousarray(a, dtype=np.float32).view(np.uint32)
    b = b + 0x7FF + ((b >> 12) & 1)
    b &= np.uint32(0xFFFFF000)
    return b.view(np.float32)


def make_in_maps(query, states, Wk, bk, Wv, bv, Wo, bo):
    """Shard the full inputs into per-core input maps (host-side prep)."""
    wv_packed = np.ascontiguousarray(
        np.transpose(Wv, (1, 0, 2)).reshape(D, H * E))
    # fold bv through the output projection: softmax rows sum to 1
    bo2 = bo.astype(np.float64).copy()
    for h in range(H):
        bo2 += bv[h].astype(np.float64) @ Wo[h * E:(h + 1) * E].astype(np.float64)
    bo2 = bo2.astype(np.float32).reshape(E, 1)
    wk_c = _round_f32r(Wk)
    wo_c = _round_f32r(Wo)
    wv_packed = _round_f32r(wv_packed)

    in_maps = []
    for b in range(B):
        in_maps.append({
            "qT": _round_f32r(query[b].T),
            "sT": _round_f32r(states[b].T),
            "wk": wk_c,
            "wv": wv_packed,
            "wo": wo_c,
            "bo2": bo2,
            "ones": np.ones((128, 128), dtype=np.float32),
        })
    return in_maps


_PROGRAM_CACHE = {}


def _get_program():
    if "nc" not in _PROGRAM_CACHE:
        _PROGRAM_CACHE["nc"] = build_program()
    return _PROGRAM_CACHE["nc"]


def kernel(query, states, Wk, bk, Wv, bv, Wo, bo, _trace=False, _tmpdir=None):
    args = [np.asarray(a, dtype=np.float32)
            for a in (query, states, Wk, bk, Wv, bv, Wo, bo)]
    nc = _get_program()
    in_maps = make_in_maps(*args)
    last_err = None
    for _attempt in range(2):  # one retry for transient device errors
        try:
            res = run_bass_kernel_spmd(nc, in_maps,
                                       core_ids=list(range(N_CORES)),
                                       trace=_trace, tmpdir=_tmpdir)
            break
        except Exception as e:  # noqa: BLE001
            last_err = e
    else:
        raise last_err
    out = np.stack([res.results[b]["outT"].T for b in range(B)])
    out = np.ascontiguousarray(out.astype(np.float32))
    if _trace:
        kernel.last_exec_time_ns = res.exec_time_ns
        kernel.last_results = res
    return out


if __name__ == "__main__":
    rng = np.random.default_rng(0)
    inputs = {
        "query": rng.standard_normal((B, LQ, E), dtype=np.float32),
        "states": rng.standard_normal((B, LK, D), dtype=np.float32),
        "Wk": rng.uniform(-0.04, 0.04, (H, D, E)).astype(np.float32),
        "bk": rng.uniform(-0.04, 0.04, (H, E)).astype(np.float32),
        "Wv": rng.uniform(-0.04, 0.04, (H, D, E)).astype(np.float32),
        "bv": rng.uniform(-0.04, 0.04, (H, E)).astype(np.float32),
        "Wo": rng.uniform(-0.015, 0.015, (H * E, E)).astype(np.float32),
        "bo": rng.uniform(-0.015, 0.015, (E,)).astype(np.float32),
    }
    out = kernel(**inputs)
    print(out.shape, out.dtype)



# revision 13
# speedup vs baseline: 1.0037x; 1.0037x over previous
"""Multi-head cross-attention kernel for Trainium2, 8 NeuronCores.

Problem: nn_MultiHeadAttention (H=32 heads, B=8, Lq=Lk=1024, E=128, D=512).

    keys   = einsum('bkd,hde->hbke', states, Wk) + bk
    values = einsum('bkd,hde->hbke', states, Wv) + bv
    attn   = softmax(einsum('bqe,hbke->hbqk', query, keys) / sqrt(E))
    ctx    = einsum('hbqk,hbke->hbqe', attn, values)  -> concat heads
    out    = ctx @ Wo + bo

Sharding: data parallel over batch B=8 -> one batch element per core; no
collectives needed.

Math restructuring (all exact algebra, host-side in fp64):
 - bk dropped: softmax(S + const-per-row) == softmax(S);
 - bv folded into the output bias: bo' = bo + sum_h bv[h] @ Wo[h];
 - Wo folded into the value projection: U[h] = Wv[h] @ Wo[h], so
   out = sum_h softmax_h @ (states @ U[h]) + bo'. This removes the whole
   output projection from the device (64 matmuls/core).

Per-core dataflow, all matmul operands bf16 (same PE rate as fp32r but
moving operand can stream N=1024, halving instruction count; rel err vs
the fp32 reference ~4e-3, well under the 2e-2 gate):

  kt[h]  = Wk[h]-chunks @ states^T        [E, Lk]  (4 accum MMs, N=1024)
  v'[g]  = states^T-blocks @ U-packed     [Lk-chunk, 8*E] (8 heads/group)
  S^T    = kt-block @ query^T             [Lk-chunk, Lq]  (1 MM, N=1024)
  P      = exp(S^T / sqrt(E))             (ACT -> bf16; scores are O(4) so
                                           exp without max-subtraction is
                                           safe in fp32)
  racc   = running sum of the 8 P chunks  (DVE bf16 adds, 2x rate)
  rowsum = ones[128,128] @ racc           [128, Lq]  (ONE matmul instead of
                                           8: the 24% of PE time the
                                           baseline spent on rowsum MMs)
  ctxO^T = v'-chunk @ P-chunks            [E, Lq]  psum accum over chunks
  out   += copy(ctxO^T) * recip(rowsum)   (DVE, pipelined one head behind
                                           so the psum tile frees early)

Steady-state per head the PE runs 25 N=1024-matmuls (~10.7us); ACT exp is
~9.2us and DVE ~9.6us, so the PE stays the bottleneck. The 8 projection
matmuls for the NEXT head/group are spread one per lk-iteration so every
exp has a >500ns window before its consumer matmul.
"""

import numpy as np
import ml_dtypes

import concourse.bass as bass  # noqa: F401  (AP helpers)
import concourse.mybir as mybir
import concourse.tile as tile
from concourse import bacc
from concourse.bass_utils import run_bass_kernel_spmd

H, E, D = 32, 128, 512
B, LQ, LK = 8, 1024, 1024
NDC = D // 128    # 4 contraction chunks for the projections
NLK = LK // 128   # 8 key chunks
HPG = 8           # heads per group for the packed V' projection
NG = H // HPG
SCALE = 1.0 / float(np.sqrt(E))

F32 = mybir.dt.float32
BF16 = mybir.dt.bfloat16
EXP = mybir.ActivationFunctionType.Exp
COPY = mybir.ActivationFunctionType.Copy

N_CORES = 8


def _build_kernel(tc, qT, sT, wk, u, ones, bo2, outT):
    nc = tc.nc
    with (
        tc.tile_pool(name="const", bufs=1) as cpool,
        tc.tile_pool(name="wkp", bufs=2) as wkp,
        tc.tile_pool(name="up", bufs=2) as upool,
        tc.tile_pool(name="ktp", bufs=2) as ktp,
        tc.tile_pool(name="vp", bufs=2) as vpool,
        tc.tile_pool(name="pp", bufs=4) as ppool,
        tc.tile_pool(name="rap", bufs=3) as rapool,
        tc.tile_pool(name="normp", bufs=2) as npool,
        tc.tile_pool(name="ps_s", bufs=2, space="PSUM") as ps_sp,
        tc.tile_pool(name="ps_c", bufs=1, space="PSUM") as ps_cp,
        tc.tile_pool(name="ps_x", bufs=1, space="PSUM") as ps_xp,
    ):
        # ---- resident inputs (st first: on the critical path) ----
        st_sb = cpool.tile([128, NDC, LK], BF16)
        for c in range(NDC):
            nc.sync.dma_start(st_sb[:, c, :], sT[c * 128:(c + 1) * 128, :])
        q_sb = cpool.tile([E, LQ], BF16)
        ones_sb = cpool.tile([128, 128], BF16)
        bo2_sb = cpool.tile([E, 1], F32)
        out_acc = cpool.tile([E, LQ], F32)

        # ---- K projection (pipelined one head ahead) ----
        kt_by_head = {}
        wk_by_head = {}
        kp_state = {}

        def emit_kproj_dma(h):
            wk_sb = wkp.tile([128, NDC, E], BF16, tag="wk", name="wk_sb")
            for c in range(NDC):
                nc.sync.dma_start(wk_sb[:, c, :],
                                  wk[h, c * 128:(c + 1) * 128, :])
            wk_by_head[h] = wk_sb

        def emit_kproj_mm(h, c):
            if c == 0:
                kp_state["ps"] = ps_xp.tile([128, LQ], F32, tag="x",
                                            name="ps_k")
            ps_k = kp_state["ps"]
            wk_sb = wk_by_head[h]
            for half in range(2):
                sl = bass.ts(half, 512)
                nc.tensor.matmul(ps_k[:, sl], wk_sb[:, c, :],
                                 st_sb[:, c, sl],
                                 start=(c == 0), stop=(c == NDC - 1))
            if c == NDC - 1:
                kt_sb = ktp.tile([E, LK], BF16, tag="kt", name="kt_sb")
                nc.vector.tensor_copy(kt_sb[:], ps_k[:])
                kt_by_head[h] = kt_sb
                del wk_by_head[h]

        # ---- V' projection (packed 8 heads; pipelined one group ahead) ----
        u_by_group = {}
        v_by_group = {}
        vp_state = {}

        def emit_u_dma(g):
            u_sb = upool.tile([128, NDC, HPG * E], BF16, tag="u", name="u_sb")
            for c in range(NDC):
                nc.sync.dma_start(
                    u_sb[:, c, :],
                    u[c * 128:(c + 1) * 128, g * HPG * E:(g + 1) * HPG * E])
            u_by_group[g] = u_sb

        def emit_vchunk_mm(g, lk, c, copy_engine=None):
            """One contraction chunk of the V' projection; at c==3 the psum
            result is copied to SBUF on `copy_engine` (ACT in steady state:
            it is idle at the head tail, and this keeps the copy off the
            DVE queue that feeds the rowsum matmul)."""
            if c == 0:
                if g not in v_by_group:
                    v_by_group[g] = vpool.tile([128, NLK, HPG * E], BF16,
                                               tag="v", name="v_sb")
                vp_state["ps"] = ps_xp.tile([128, LQ], F32, tag="x",
                                            name="ps_v")
            ps_v = vp_state["ps"]
            for half in range(2):
                sl = bass.ts(half, 512)
                nc.tensor.matmul(ps_v[:, sl],
                                 st_sb[:, c, lk * 128:(lk + 1) * 128],
                                 u_by_group[g][:, c, sl],
                                 start=(c == 0), stop=(c == NDC - 1))
            if c == NDC - 1:
                dst = v_by_group[g][:, lk, :]
                if copy_engine == "act":
                    nc.scalar.activation(dst, ps_v[:], COPY)
                else:
                    nc.vector.tensor_copy(dst, ps_v[:])

        # ---- rowsum + normalization, pipelined one head behind ----
        pending_tail = {}   # h -> racc (feeds the rowsum matmul)
        pending_norm = {}   # h -> (ctx_raw, recip)

        def emit_rowsum_prev():
            """Previous head's rowsum matmul + reciprocal (emitted after the
            next head's first S matmul so the PE queue never blocks on the
            DVE tail chain)."""
            if not pending_tail:
                return
            (h, (racc, ctx_raw)), = pending_tail.items()
            pending_tail.clear()
            ps_r = ps_xp.tile([128, LQ], F32, tag="x", name="ps_r")
            for half in range(2):
                sl = bass.ts(half, 512)
                nc.tensor.matmul(ps_r[:, sl], ones_sb[:], racc[:, sl],
                                 start=True, stop=True)
            recip_sb = npool.tile([128, LQ], F32, tag="recip",
                                  name="recip_sb")
            nc.vector.reciprocal_approx_fast(recip_sb[:], ps_r[:])
            pending_norm[h] = (ctx_raw, recip_sb)

        def emit_norm():
            if not pending_norm:
                return
            (h, (ctx_raw, recip_sb)), = pending_norm.items()
            pending_norm.clear()
            ctxn = npool.tile([E, LQ], F32, tag="ctxn", name="ctxn")
            nc.vector.tensor_mul(ctxn[:], ctx_raw[:], recip_sb[:])
            if h == 0:
                nc.vector.tensor_scalar_add(out_acc[:], ctxn[:],
                                            bo2_sb[:, 0:1])
            else:
                nc.gpsimd.tensor_add(out_acc[:], out_acc[:], ctxn[:])

        # ---- prologue ----
        emit_kproj_dma(0)
        emit_u_dma(0)
        nc.sync.dma_start(q_sb[:], qT[:])
        nc.sync.dma_start(ones_sb[:], ones[:])
        nc.sync.dma_start(bo2_sb[:], bo2[:])
        for c in range(NDC):
            emit_kproj_mm(0, c)
        for c in range(NDC):
            emit_vchunk_mm(0, 0, c)
        for c in range(NDC):
            emit_vchunk_mm(0, 1, c)
        emit_kproj_dma(1)
        emit_u_dma(1)

        # ---- head loop ----
        for h in range(H):
            g, hh = divmod(h, HPG)
            kt_sb = kt_by_head.pop(h)
            v_sb = v_by_group[g]

            def emit_s(lk, kt_sb=kt_sb):
                ps_s = ps_sp.tile([128, LQ], F32, tag="s", name="ps_s")
                for half in range(2):
                    sl = bass.ts(half, 512)
                    nc.tensor.matmul(ps_s[:, sl],
                                     kt_sb[:, lk * 128:(lk + 1) * 128],
                                     q_sb[:, sl], start=True, stop=True)
                p_sb = ppool.tile([128, LQ], BF16, tag="p", name="p_sb")
                nc.scalar.activation(p_sb[:], ps_s[:], EXP, scale=SCALE)
                return p_sb

            ps_c = ps_cp.tile([E, LQ], F32, tag="c", name="ps_c")
            p_next = emit_s(0)
            emit_rowsum_prev()
            racc = None
            for lk in range(NLK):
                p_cur = p_next
                if lk + 1 < NLK:
                    p_next = emit_s(lk + 1)
                # AV accumulation first at lk==0 (its input exp is the
                # oldest); projection matmuls fill the PE while ACT works
                # on this iteration's exp
                if lk == 0:
                    for half in range(2):
                        sl = bass.ts(half, 512)
                        nc.tensor.matmul(ps_c[:, sl],
                                         v_sb[:, lk, hh * E:(hh + 1) * E],
                                         p_cur[:, sl], start=True, stop=False)
                # one projection matmul per iteration in steady state
                if h == 0:
                    # bootstrap: group 0's remaining V' chunks JIT (bursts),
                    # then next head's K, then group 1's first chunk
                    if lk < 6:
                        for c in range(NDC):
                            emit_vchunk_mm(0, lk + 2, c)
                    elif lk == 6:
                        for c in range(NDC):
                            emit_kproj_mm(1, c)
                    else:
                        for c in range(NDC):
                            emit_vchunk_mm(1, 0, c, copy_engine="act")
                else:
                    if lk < NDC:
                        if h + 1 < H:
                            emit_kproj_mm(h + 1, lk)
                    else:
                        # heads 1..7 cover group 1 chunks 1..7 (chunk 0 was
                        # done by head 0); heads (g>=1,hh) cover chunk hh
                        # of group g+1
                        c = lk - NDC
                        if g + 1 < NG and not (g == 0 and hh == 0):
                            emit_vchunk_mm(g + 1, hh, c, copy_engine="act")
                if lk == 2:
                    emit_norm()
                if lk == 0 and h + 2 < H:
                    emit_kproj_dma(h + 2)
                if lk == 1 and hh == 0 and g + 2 < NG:
                    emit_u_dma(g + 2)
                if lk > 0:
                    for half in range(2):
                        sl = bass.ts(half, 512)
                        nc.tensor.matmul(ps_c[:, sl],
                                         v_sb[:, lk, hh * E:(hh + 1) * E],
                                         p_cur[:, sl], start=False,
                                         stop=(lk == NLK - 1))
                # rowsum running accumulation on DVE (bf16, 2x rate)
                if lk == 0:
                    racc = p_cur
                else:
                    racc_new = rapool.tile([128, LQ], BF16, tag="racc",
                                           name="racc")
                    nc.vector.tensor_add(racc_new[:], racc[:], p_cur[:])
                    racc = racc_new

            # ---- tail: raw-ctx copy frees ps_c; rowsum is deferred to the
            # next head's first iteration ----
            ctx_raw = npool.tile([E, LQ], BF16, tag="ctx", name="ctx_raw")
            nc.vector.tensor_copy(ctx_raw[:], ps_c[:])
            pending_tail[h] = (racc, ctx_raw)

        emit_rowsum_prev()
        emit_norm()
        nc.sync.dma_start(outT[:], out_acc[:])


def build_program():
    nc = bacc.Bacc("TRN2", target_bir_lowering=False, debug=False,
                   num_devices=N_CORES)
    qT = nc.dram_tensor("qT", [E, LQ], BF16, kind="ExternalInput").ap()
    sT = nc.dram_tensor("sT", [D, LK], BF16, kind="ExternalInput").ap()
    wk = nc.dram_tensor("wk", [H, D, E], BF16, kind="ExternalInput").ap()
    u = nc.dram_tensor("u", [D, H * E], BF16, kind="ExternalInput").ap()
    ones = nc.dram_tensor("ones", [128, 128], BF16, kind="ExternalInput").ap()
    bo2 = nc.dram_tensor("bo2", [E, 1], F32, kind="ExternalInput").ap()
    outT = nc.dram_tensor("outT", [E, LQ], F32, kind="ExternalOutput").ap()

    with tile.TileContext(nc) as tc:
        _build_kernel(tc, qT, sT, wk, u, ones, bo2, outT)
    nc.compile()
    return nc


def make_in_maps(query, states, Wk, bk, Wv, bv, Wo, bo):
    """Shard the full inputs into per-core input maps (host-side prep)."""
    bb = ml_dtypes.bfloat16
    WoH = Wo.reshape(H, E, E).astype(np.float64)
    # fold Wo through the value projection and bv through the output bias
    # (softmax rows sum to 1), both exact in fp64
    U = np.einsum('hde,hef->hdf', Wv.astype(np.float64), WoH)
    u_packed = np.ascontiguousarray(
        np.transpose(U, (1, 0, 2)).reshape(D, H * E)).astype(bb)
    bo2 = bo.astype(np.float64) + np.einsum('he,hef->f',
                                            bv.astype(np.float64), WoH)
    bo2 = bo2.astype(np.float32).reshape(E, 1)
    wk_c = np.ascontiguousarray(Wk).astype(bb)
    ones_c = np.ones((128, 128), dtype=bb)

    in_maps = []
    for b in range(B):
        in_maps.append({
            "qT": np.ascontiguousarray(query[b].T).astype(bb),
            "sT": np.ascontiguousarray(states[b].T).astype(bb),
            "wk": wk_c,
            "u": u_packed,
            "ones": ones_c,
            "bo2": bo2,
        })
    return in_maps


_PROGRAM_CACHE = {}


def _get_program():
    if "nc" not in _PROGRAM_CACHE:
        _PROGRAM_CACHE["nc"] = build_program()
    return _PROGRAM_CACHE["nc"]


def kernel(query, states, Wk, bk, Wv, bv, Wo, bo, _trace=False, _tmpdir=None):
    args = [np.asarray(a, dtype=np.float32)
            for a in (query, states, Wk, bk, Wv, bv, Wo, bo)]
    nc = _get_program()
    in_maps = make_in_maps(*args)
    last_err = None
    for _attempt in range(2):  # one retry for transient device errors
        try:
            res = run_bass_kernel_spmd(nc, in_maps,
                                       core_ids=list(range(N_CORES)),
                                       trace=_trace, tmpdir=_tmpdir)
            break
        except Exception as e:  # noqa: BLE001
            last_err = e
    else:
        raise last_err
    out = np.stack([res.results[b]["outT"].T for b in range(B)])
    out = np.ascontiguousarray(out.astype(np.float32))
    if _trace:
        kernel.last_exec_time_ns = res.exec_time_ns
        kernel.last_results = res
    return out


if __name__ == "__main__":
    rng = np.random.default_rng(0)
    inputs = {
        "query": rng.standard_normal((B, LQ, E), dtype=np.float32),
        "states": rng.standard_normal((B, LK, D), dtype=np.float32),
        "Wk": rng.uniform(-0.04, 0.04, (H, D, E)).astype(np.float32),
        "bk": rng.uniform(-0.04, 0.04, (H, E)).astype(np.float32),
        "Wv": rng.uniform(-0.04, 0.04, (H, D, E)).astype(np.float32),
        "bv": rng.uniform(-0.04, 0.04, (H, E)).astype(np.float32),
        "Wo": rng.uniform(-0.015, 0.015, (H * E, E)).astype(np.float32),
        "bo": rng.uniform(-0.015, 0.015, (E,)).astype(np.float32),
    }
    out = kernel(**inputs)
    print(out.shape, out.dtype)


# revision 18
# speedup vs baseline: 1.1519x; 1.1476x over previous
"""Multi-head cross-attention kernel for Trainium2, 8 NeuronCores.

Problem: nn_MultiHeadAttention (H=32 heads, B=8, Lq=Lk=1024, E=128, D=512).

    keys   = einsum('bkd,hde->hbke', states, Wk) + bk
    values = einsum('bkd,hde->hbke', states, Wv) + bv
    attn   = softmax(einsum('bqe,hbke->hbqk', query, keys) / sqrt(E))
    ctx    = einsum('hbqk,hbke->hbqe', attn, values)  -> concat heads
    out    = ctx @ Wo + bo

Sharding: data parallel over batch B=8 -> one batch element per core; no
collectives needed.

Math restructuring (exact algebra, host-side in fp64):
 - bk dropped: softmax(S + const-per-row) == softmax(S);
 - bv folded into the output bias: bo' = bo + sum_h bv[h] @ Wo[h];
 - Wo folded into the value projection: U[h] = Wv[h] @ Wo[h], so
   out = sum_h softmax_h @ (states @ U[h]) + bo'. This removes the output
   projection matmuls from the device entirely.

Per-core dataflow (matmul moving operands always N=512 at full PE rate):

  kt[h]  = Wk[h]-chunks @ states^T     [E, Lk]   (wk bf16 stationary)
  v'[4h] = states^T-blocks @ U-packed  [Lk-chunk, 4*E]  (4 heads at once)
  S^T    = kt-block @ query^T          [Lk-chunk, Lq]
  P      = exp(S^T / sqrt(E)) -> bf16  (ACT; scores are O(4) so exp without
                                        max-subtraction is safe in fp32)
  racc   = running sum of the 8 P chunks (bf16 adds: 2 on GpSimd early,
                                        5 on DVE; DVE bf16 adds hit the
                                        2x packed mode, ~830ns each)
  rowsum = ones[128,128] @ racc        ONE psum tile (2 half-matmuls)
                                        instead of the baseline's 16: the
                                        24% of PE time rowsum used to take
  ctx^T  = v'-chunk @ P-chunks         [E, Lq] psum accum over chunks
  out   += copy(ctx^T) * recip(rowsum) (copy/recip/mul on DVE, final
                                        accumulate on GpSimd, all pipelined
                                        one head behind)

The rowsum matmul + reciprocal of head h are emitted at head h+1's start,
the normalization multiply at its lk==2, and the output accumulation on
GpSimd behind that, so no engine's FIFO ever blocks the PE queue.
"""

import numpy as np
import ml_dtypes

import concourse.bass as bass
import concourse.mybir as mybir
import concourse.tile as tile
from concourse import bacc
from concourse.bass_utils import run_bass_kernel_spmd

H, E, D = 32, 128, 512
B, LQ, LK = 8, 1024, 1024
NDC = D // 128    # 4 contraction chunks for the projections
NLK = LK // 128   # 8 key chunks
HPG = 4           # heads per group for the packed V' computation
NG = H // HPG
SCALE = 1.0 / float(np.sqrt(E))

F32 = mybir.dt.float32
F32R = mybir.dt.float32r
BF16 = mybir.dt.bfloat16
EXP = mybir.ActivationFunctionType.Exp
COPY = mybir.ActivationFunctionType.Copy

N_CORES = 8


def _round_f32r(a):
    """Round fp32 -> fp32r (11-bit mantissa, low 12 bits zero), RN-even."""
    b = np.ascontiguousarray(a, dtype=np.float32).view(np.uint32)
    b = b + 0x7FF + ((b >> 12) & 1)
    b &= np.uint32(0xFFFFF000)
    return b.view(np.float32)


def _build_kernel(tc, qT, sT, wk, u, ones, bo2, outT):
    nc = tc.nc
    with (
        tc.tile_pool(name="const", bufs=1) as cpool,
        tc.tile_pool(name="wkp", bufs=2) as wkp,
        tc.tile_pool(name="up", bufs=2) as upool,
        tc.tile_pool(name="ktp", bufs=2) as ktp,
        tc.tile_pool(name="vp", bufs=2) as vpool,
        tc.tile_pool(name="pp", bufs=4) as ppool,
        tc.tile_pool(name="rap", bufs=3) as rapool,
        tc.tile_pool(name="normp", bufs=2) as npool,
        tc.tile_pool(name="ps_sh", bufs=2, space="PSUM") as ps_sh,
        tc.tile_pool(name="ps_acc", bufs=1, space="PSUM") as ps_acc,
    ):
        # ---- resident inputs; st chunks are separate tiles so the first
        # projection matmul only waits on the first quarter of the DMA ----
        st_sb = [cpool.tile([128, LK], BF16, name=f"st{c}")
                 for c in range(NDC)]
        nc.sync.dma_start(st_sb[0][:], sT[0:128, :])
        q_sb = cpool.tile([E, LQ], F32R)
        ones_sb = cpool.tile([128, 128], BF16)
        bo2_sb = cpool.tile([E, 1], F32)
        out_acc = cpool.tile([E, LQ], F32)

        kt_by_head = {}
        wk_by_head = {}

        def emit_k_dma(h):
            wk_sb = wkp.tile([128, NDC, E], BF16, tag="wk", name="wk_sb")
            for c in range(NDC):
                nc.sync.dma_start(wk_sb[:, c, :],
                                  wk[h, c * 128:(c + 1) * 128, :])
            wk_by_head[h] = wk_sb

        def emit_k(h):
            """K^T projection for head h (pipelined one head ahead).
            One [E, LK] psum tile, single DVE copy (ACT is exp-bound)."""
            wk_sb = wk_by_head.pop(h)
            kt_sb = ktp.tile([E, LK], F32R, tag="kt", name="kt_sb")
            ps_k = ps_sh.tile([E, LQ], F32, tag="sh", name="ps_k")
            for half in range(2):
                sl = bass.ts(half, 512)
                for c in range(NDC):
                    nc.tensor.matmul(ps_k[:, sl], wk_sb[:, c, :],
                                     st_sb[c][:, sl],
                                     start=(c == 0), stop=(c == NDC - 1))
            nc.vector.tensor_copy(kt_sb[:], ps_k[:])
            kt_by_head[h] = kt_sb

        # ---- rowsum + normalization, pipelined one head behind ----
        pending_tail = {}   # h -> (racc, ctx_raw)
        pending_norm = {}   # h -> (ctx_raw, recip)

        def emit_rowsum_prev():
            if not pending_tail:
                return
            (h, (racc, ctx_raw)), = pending_tail.items()
            pending_tail.clear()
            ps_r = ps_acc.tile([128, LQ], F32, tag="r", name="ps_r")
            for half in range(2):
                sl = bass.ts(half, 512)
                nc.tensor.matmul(ps_r[:, sl], ones_sb[:], racc[:, sl],
                                 start=True, stop=True)
            recip_sb = npool.tile([128, LQ], F32, tag="recip",
                                  name="recip_sb")
            nc.vector.reciprocal_approx_fast(recip_sb[:], ps_r[:])
            pending_norm[h] = (ctx_raw, recip_sb)

        def emit_norm(last=False):
            if not pending_norm:
                return
            (h, (ctx_raw, recip_sb)), = pending_norm.items()
            pending_norm.clear()
            ctxn = npool.tile([E, LQ], F32, tag="ctxn", name="ctxn")
            nc.vector.tensor_mul(ctxn[:], ctx_raw[:], recip_sb[:])
            if h == 0:
                nc.vector.tensor_scalar_add(out_acc[:], ctxn[:],
                                            bo2_sb[:, 0:1])
            elif last:
                # final head: keep the tail latency off GpSimd and overlap
                # the output DMA half by half
                for half in range(2):
                    sl = bass.ts(half, 512)
                    nc.vector.tensor_add(out_acc[:, sl], out_acc[:, sl],
                                         ctxn[:, sl])
                    nc.sync.dma_start(outT[:, sl], out_acc[:, sl])
            else:
                nc.gpsimd.tensor_add(out_acc[:], out_acc[:], ctxn[:])

        # ---- prologue ----
        emit_k_dma(0)
        for c in range(1, NDC):
            nc.sync.dma_start(st_sb[c][:], sT[c * 128:(c + 1) * 128, :])
        u_sb0 = upool.tile([128, NDC, HPG * E], BF16, tag="u", name="u_sb")
        for c in range(NDC):
            nc.sync.dma_start(u_sb0[:, c, :], u[c * 128:(c + 1) * 128,
                                                0:HPG * E])
        nc.sync.dma_start(q_sb[:], qT[:])
        nc.sync.dma_start(ones_sb[:], ones[:])
        nc.sync.dma_start(bo2_sb[:], bo2[:])
        emit_k(0)
        emit_k_dma(1)

        vstate = {}

        def emit_vchunk(g, lk):
            """One lk-chunk of group g's packed V' projection (4 matmuls +
            ACT copy). Chunks are spread through the head loop to keep the
            PE ahead of the exp chain on otherwise-thin iterations."""
            ps_v = ps_sh.tile([128, HPG * E], F32, tag="sh", name="ps_v")
            for c in range(NDC):
                nc.tensor.matmul(
                    ps_v[:], st_sb[c][:, lk * 128:(lk + 1) * 128],
                    vstate[("u", g)][:, c, :],
                    start=(c == 0), stop=(c == NDC - 1))
            nc.scalar.activation(vstate[("v", g)][:, lk, :], ps_v[:], COPY)

        def prep_group(g, u_sb=None):
            if u_sb is None:
                u_sb = upool.tile([128, NDC, HPG * E], BF16, tag="u",
                                  name="u_sb")
                for c in range(NDC):
                    nc.sync.dma_start(
                        u_sb[:, c, :],
                        u[c * 128:(c + 1) * 128,
                          g * HPG * E:(g + 1) * HPG * E])
            vstate[("u", g)] = u_sb
            vstate[("v", g)] = vpool.tile([128, NLK, HPG * E], BF16,
                                          tag="v", name="v_sb")

        prep_group(0, u_sb0)
        for lk in range(2):
            emit_vchunk(0, lk)

        for g in range(NG):
            v_sb = vstate[("v", g)]
            if g + 1 < NG and g == 0:
                prep_group(g + 1)

            for hh in range(HPG):
                h = g * HPG + hh
                kt_sb = kt_by_head.pop(h)

                def emit_s(lk, kt_sb=kt_sb):
                    ps_s = ps_sh.tile([128, LQ], F32, tag="sh", name="ps_s")
                    for half in range(2):
                        sl = bass.ts(half, 512)
                        nc.tensor.matmul(ps_s[:, sl],
                                         kt_sb[:, lk * 128:(lk + 1) * 128],
                                         q_sb[:, sl], start=True, stop=True)
                    p_sb = ppool.tile([128, LQ], BF16, tag="p", name="p_sb")
                    nc.scalar.activation(p_sb[:], ps_s[:], EXP, scale=SCALE)
                    return p_sb

                ps_c = ps_acc.tile([E, LQ], F32, tag="c", name="ps_c")
                p_next = emit_s(0)
                emit_rowsum_prev()
                racc = None
                for lk in range(NLK):
                    p_cur = p_next
                    if lk + 1 < NLK:
                        p_next = emit_s(lk + 1)
                    if lk == 0 and h + 1 < H:
                        emit_k(h + 1)
                        if h + 2 < H:
                            emit_k_dma(h + 2)
                    if lk == 1 and hh == 0 and g + 1 < NG and g > 0:
                        prep_group(g + 1)
                    if lk == 2:
                        emit_norm()
                    # V' chunks for the next group fill the thin iterations
                    # (head 0 JIT-computes its own group's chunks instead)
                    if h == 0:
                        if lk < 6:
                            emit_vchunk(0, lk + 2)
                        else:
                            emit_vchunk(1, lk - 6)
                    elif g + 1 < NG and lk in (4, 6):
                        if not (hh == 0 and g == 0):
                            emit_vchunk(g + 1, 2 * hh + (lk - 4) // 2)
                    for half in range(2):
                        sl = bass.ts(half, 512)
                        nc.tensor.matmul(ps_c[:, sl],
                                         v_sb[:, lk, hh * E:(hh + 1) * E],
                                         p_cur[:, sl],
                                         start=(lk == 0), stop=(lk == NLK - 1))
                    # rowsum running accumulation: first two adds on GpSimd
                    # (slow but early -- the DVE chain catches up by lk 7,
                    # keeping the post-exp7 tail to one ~830ns DVE add)
                    if lk == 0:
                        racc = p_cur
                    else:
                        racc_new = rapool.tile([128, LQ], BF16, tag="racc",
                                               name="racc")
                        eng = nc.gpsimd if lk <= 2 else nc.vector
                        eng.tensor_add(racc_new[:], racc[:], p_cur[:])
                        racc = racc_new

                # tail: free ps_c early; rowsum/norm happen next head
                ctx_raw = npool.tile([E, LQ], BF16, tag="ctx", name="ctx_raw")
                nc.vector.tensor_copy(ctx_raw[:], ps_c[:])
                pending_tail[h] = (racc, ctx_raw)

        emit_rowsum_prev()
        emit_norm(last=True)


def build_program():
    nc = bacc.Bacc("TRN2", target_bir_lowering=False, debug=False,
                   num_devices=N_CORES)
    qT = nc.dram_tensor("qT", [E, LQ], F32R, kind="ExternalInput").ap()
    sT = nc.dram_tensor("sT", [D, LK], BF16, kind="ExternalInput").ap()
    wk = nc.dram_tensor("wk", [H, D, E], BF16, kind="ExternalInput").ap()
    u = nc.dram_tensor("u", [D, H * E], BF16, kind="ExternalInput").ap()
    ones = nc.dram_tensor("ones", [128, 128], BF16, kind="ExternalInput").ap()
    bo2 = nc.dram_tensor("bo2", [E, 1], F32, kind="ExternalInput").ap()
    outT = nc.dram_tensor("outT", [E, LQ], F32, kind="ExternalOutput").ap()

    with tile.TileContext(nc) as tc:
        _build_kernel(tc, qT, sT, wk, u, ones, bo2, outT)
    nc.compile()
    return nc


def make_in_maps(query, states, Wk, bk, Wv, bv, Wo, bo):
    """Shard the full inputs into per-core input maps (host-side prep)."""
    bb = ml_dtypes.bfloat16
    WoH = Wo.reshape(H, E, E).astype(np.float64)
    # fold Wo through the value projection and bv through the output bias
    # (softmax rows sum to 1), both exact in fp64
    U = np.einsum('hde,hef->hdf', Wv.astype(np.float64), WoH)
    u_packed = np.ascontiguousarray(
        np.transpose(U, (1, 0, 2)).reshape(D, H * E)).astype(bb)
    bo2 = bo.astype(np.float64) + np.einsum('he,hef->f',
                                            bv.astype(np.float64), WoH)
    bo2 = bo2.astype(np.float32).reshape(E, 1)
    wk_c = np.ascontiguousarray(Wk).astype(bb)
    ones_c = np.ones((128, 128), dtype=bb)

    in_maps = []
    for b in range(B):
        in_maps.append({
            "qT": _round_f32r(query[b].T),
            "sT": np.ascontiguousarray(states[b].T).astype(bb),
            "wk": wk_c,
            "u": u_packed,
            "ones": ones_c,
            "bo2": bo2,
        })
    return in_maps


_PROGRAM_CACHE = {}


def _get_program():
    if "nc" not in _PROGRAM_CACHE:
        _PROGRAM_CACHE["nc"] = build_program()
    return _PROGRAM_CACHE["nc"]


def kernel(query, states, Wk, bk, Wv, bv, Wo, bo, _trace=False, _tmpdir=None):
    args = [np.asarray(a, dtype=np.float32)
            for a in (query, states, Wk, bk, Wv, bv, Wo, bo)]
    nc = _get_program()
    in_maps = make_in_maps(*args)
    last_err = None
    for _attempt in range(2):  # one retry for transient device errors
        try:
            res = run_bass_kernel_spmd(nc, in_maps,
                                       core_ids=list(range(N_CORES)),
                                       trace=_trace, tmpdir=_tmpdir)
            break
        except Exception as e:  # noqa: BLE001
            last_err = e
    else:
        raise last_err
    out = np.stack([res.results[b]["outT"].T for b in range(B)])
    out = np.ascontiguousarray(out.astype(np.float32))
    if _trace:
        kernel.last_exec_time_ns = res.exec_time_ns
        kernel.last_results = res
    return out


if __name__ == "__main__":
    rng = np.random.default_rng(0)
    inputs = {
        "query": rng.standard_normal((B, LQ, E), dtype=np.float32),
        "states": rng.standard_normal((B, LK, D), dtype=np.float32),
        "Wk": rng.uniform(-0.04, 0.04, (H, D, E)).astype(np.float32),
        "bk": rng.uniform(-0.04, 0.04, (H, E)).astype(np.float32),
        "Wv": rng.uniform(-0.04, 0.04, (H, D, E)).astype(np.float32),
        "bv": rng.uniform(-0.04, 0.04, (H, E)).astype(np.float32),
        "Wo": rng.uniform(-0.015, 0.015, (H * E, E)).astype(np.float32),
        "bo": rng.uniform(-0.015, 0.015, (E,)).astype(np.float32),
    }
    out = kernel(**inputs)
    print(out.shape, out.dtype)
